# revision 17
# baseline (speedup 1.0000x reference)
"""Depthwise causal Conv1d (K=16) for x:(4, 2048, 8192) f32 on 8 TRN2 NeuronCores.

Strategy (tensor-parallel over channels, no cross-core communication):
  - Each core owns 256 channels (2048 / 8) for all 4 batches.
  - The time axis is cut into 112-output windows; window j of one channel
    is a banded-Toeplitz matmul on the TensorEngine:
        psum[m, (b,j)] = sum_p A[p, m] * X[p, (b,j)]
        A[p, m]   = w[126 - p - m]              for 111 <= p + m <= 126
        X[p, (b,j)] = x[b, c, 112*j + 111 - p]  (zero for t < 0)
    Only NJ=73 full windows run on device (outputs [0, 8176)); the last 16
    samples, the first 15 samples (see below) and the (identically zero)
    bias are computed on the host, so no padded junk window is shipped.
  - The problem is HBM-bandwidth bound (16 DMA engines x 25.6 GB/s per
    core), so all DRAM traffic is bf16 (~0.4% rel err vs the 2e-2 budget)
    and the 15-sample causal halo is deduplicated for the first 128
    channels per core ("dense mode"): x ships as 112 dense rows only, and
    the halo contribution is a second accumulating matmul (contraction 16:
    15 halo taps + a zero pad row) whose moving operand is rows 0..15 of
    the PREVIOUS window column, already in SBUF via a shifted AP.  Column
    j=0 of each batch then accumulates garbage into outputs m<15 (its
    "previous column" belongs to another batch); those land in y[:,:,0:15]
    which the host recomputes anyway.  The remaining 128 channels ship the
    classic 128-row windows (halo from HBM) to keep the TensorEngine under
    the DMA roofline.
  - DMA queues are dedicated: GpSimd = all loads (pure prefetch), Sync =
    all stores (compute-dependent), so prefetch never queues behind a
    store.  A single consolidated load queue measurably beats two.  Last
    chunks are tapered (8 + 8) to shorten the drain.
  - Epilogue: PSUM(f32) -> SBUF(bf16) converting copies, two channels per
    instruction, alternating Vector / Scalar engines.

The host does the sharding + window-layout transposes with numpy; the device
kernel sees only dense p-major arrays.
"""

import os
import sys

import numpy as np
from numpy.lib.stride_tricks import sliding_window_view

if "/opt/trn_rl_repo" not in sys.path:
    sys.path.insert(0, "/opt/trn_rl_repo")

import ml_dtypes

import concourse.bacc as bacc
import concourse.mybir as mybir
import concourse.tile as tile
from concourse.bass_utils import run_bass_kernel_spmd

F32 = mybir.dt.float32
BF16 = mybir.dt.bfloat16
NP_BF16 = np.dtype(ml_dtypes.bfloat16)
ACT_COPY = mybir.ActivationFunctionType.Copy

N_CORES = 8
B = 4             # batch
DIM = 2048        # channels
T = 8192          # time
K = 16            # conv taps
C = DIM // N_CORES    # channels per core = 256
PIN = 128         # windowed-mode contraction rows (127-sample window + 0 row)
PD = 112          # dense-mode contraction rows / outputs per window
PH = 16           # dense-mode halo contraction rows (15 taps + 0 row)
PO = 112          # outputs per window, multiple of 16
NJ = 73           # full windows per (batch, channel); edges done on host
TD = NJ * PO          # device-computed samples per channel = 8176
XC = B * NJ           # x / out cols per channel = 292
PSB = 512             # psum bank stride (f32 elems); channel pair at 0 / PSB
CD = 128          # dense-mode channels per core (local channels [0, CD))
DCHUNKS = [16] * 8            # dense-mode chunks
WCHUNKS = [16] * 7 + [8, 8]   # windowed-mode chunks (tapered tail)

_compiled_nc = None


def _build_kernel():
    nc = bacc.Bacc(None)

    xin_d = nc.declare_dram_parameter("xin_d", [PD, CD, XC], BF16, isOutput=False)
    a0_d = nc.declare_dram_parameter("a0_d", [PD, CD, PO], BF16, isOutput=False)
    a1_d = nc.declare_dram_parameter("a1_d", [PH, CD, PH], BF16, isOutput=False)
    xin_w = nc.declare_dram_parameter("xin_w", [PIN, C - CD, XC], BF16, isOutput=False)
    a_w = nc.declare_dram_parameter("a_w", [PIN, C - CD, PO], BF16, isOutput=False)
    yout = nc.declare_dram_parameter("yout", [PO, C, XC], BF16, isOutput=True)

    with tile.TileContext(nc) as tc:
        with (
            tc.tile_pool(name="xpool", bufs=7) as xpool,
            tc.tile_pool(name="apool", bufs=7) as apool,
            tc.tile_pool(name="a1pool", bufs=4) as a1pool,
            tc.tile_pool(name="opool", bufs=6) as opool,
            tc.tile_pool(name="psum", bufs=4, space="PSUM") as pspool,
        ):
            c0 = 0
            for ci, ch in enumerate(DCHUNKS + WCHUNKS):
                dense = ci < len(DCHUNKS)
                n = ch * XC
                hf = ch // 2          # channels per half
                nh = hf * XC          # cols per half
                a_t = apool.tile([PD if dense else PIN, ch * PO], BF16)
                o_t = opool.tile([PO, ch * XC], BF16)

                if dense:
                    cl = c0                       # dense-local == core-local
                    x_t = xpool.tile([PD, n], BF16)
                    a1_t = a1pool.tile([PH, ch * PH], BF16)
                    nc.gpsimd.dma_start(
                        out=x_t[:].rearrange("p (c j) -> p c j", c=ch),
                        in_=xin_d[:, cl : cl + ch, :],
                    )
                    nc.gpsimd.dma_start(
                        out=a_t[:].rearrange("p (c m) -> p c m", c=ch),
                        in_=a0_d[:, cl : cl + ch, :],
                    )
                    nc.gpsimd.dma_start(
                        out=a1_t[:].rearrange("p (c m) -> p c m", c=ch),
                        in_=a1_d[:, cl : cl + ch, :],
                    )
                else:
                    cl = c0 - CD                  # windowed-local channel
                    x_t = xpool.tile([PIN, n], BF16)
                    nc.gpsimd.dma_start(
                        out=x_t[:].rearrange("p (c j) -> p c j", c=ch),
                        in_=xin_w[:, cl : cl + ch, :],
                    )
                    nc.gpsimd.dma_start(
                        out=a_t[:].rearrange("p (c m) -> p c m", c=ch),
                        in_=a_w[:, cl : cl + ch, :],
                    )

                for g in range(ch // 2):
                    ps = pspool.tile([PO, 2 * PSB], F32)
                    for h in range(2):
                        i = 2 * g + h
                        nc.tensor.matmul(
                            ps[:, h * PSB : h * PSB + XC],
                            a_t[:, i * PO : (i + 1) * PO],
                            x_t[:, i * XC : (i + 1) * XC],
                            start=True,
                            stop=not dense,
                        )
                        if dense:
                            # halo taps from the previous window column
                            nc.tensor.matmul(
                                ps[0:PH, h * PSB + 1 : h * PSB + XC],
                                a1_t[:, i * PH : (i + 1) * PH],
                                x_t[0:PH, i * XC : (i + 1) * XC - 1],
                                start=False,
                                stop=True,
                            )
                    # converting psum(f32) -> sbuf(bf16) copy, 2 ch per inst
                    src = ps[:].rearrange("p (g q) -> p g q", g=2)[:, :, 0:XC]
                    dst = o_t[:, 2 * g * XC : (2 * g + 2) * XC].rearrange(
                        "p (g q) -> p g q", g=2
                    )
                    if g % 2 == 0:
                        nc.vector.tensor_copy(dst, src)
                    else:
                        nc.scalar.activation(dst, src, ACT_COPY)

                    # store (Sync queue) as soon as each half's copies land
                    if 2 * (g + 1) == hf:
                        nc.sync.dma_start(
                            out=yout[:, c0 : c0 + hf, :],
                            in_=o_t[:, 0:nh].rearrange("p (c j) -> p c j", c=hf),
                        )
                    elif 2 * (g + 1) == ch:
                        nc.sync.dma_start(
                            out=yout[:, c0 + hf : c0 + ch, :],
                            in_=o_t[:, nh:n].rearrange("p (c j) -> p c j", c=hf),
                        )
                c0 += ch

    nc.compile()
    return nc


def _get_nc():
    global _compiled_nc
    if _compiled_nc is None:
        _compiled_nc = _build_kernel()
    return _compiled_nc


def _prep_core(x, weight, bias, core):
    """Build the per-core input map (numpy only)."""
    cs = slice(core * C, (core + 1) * C)
    xs = x[:, cs, :]                       # [B, C, T]
    w = weight[cs, 0, :]                   # [C, K]

    # band matrix for all channels: A[c, p, m] = w[c, 126 - p - m] masked
    idx = np.arange(PIN - 1)[:, None] + np.arange(PO)[None, :]   # p + m
    amask = (idx >= 111) & (idx <= 126)
    aidx = np.clip(126 - idx, 0, K - 1)
    a_mat = np.where(amask[None], w[:, aidx], 0.0)               # [C, 127, PO]

    # dense-mode channels [0, CD): X[p, c, (b,j)] = x[b, c, 112j + 111 - p]
    blocks = xs[:, :CD, :TD].reshape(B, CD, NJ, PD)[:, :, :, ::-1]
    xin_d = np.ascontiguousarray(blocks.transpose(3, 1, 0, 2))   # [112,CD,B,NJ]
    xin_d = xin_d.reshape(PD, CD, XC).astype(NP_BF16)
    a0_d = np.ascontiguousarray(
        a_mat[:CD, 0:PD, :].transpose(1, 0, 2)
    ).astype(NP_BF16)                                            # [112, CD, PO]
    a1_d = np.zeros((PH, CD, PH), dtype=np.float32)              # [16, CD, 16]
    a1_d[0:15, :, 0:15] = a_mat[:CD, PD : PIN - 1, 0:15].transpose(1, 0, 2)
    a1_d = np.ascontiguousarray(a1_d).astype(NP_BF16)

    # windowed-mode channels [CD, C): 127-sample windows + zero row 127
    CW = C - CD
    xw = xs[:, CD:, :]
    PW = PIN - 1
    xpad = np.zeros((B, CW, PO * (NJ - 1) + PW), dtype=np.float32)
    xpad[:, :, K - 1 :] = xw[:, :, :TD]
    sw = sliding_window_view(xpad, PW, axis=2)[:, :, ::PO, :]    # [B,CW,NJ,127]
    xin_w = np.zeros((PIN, CW, B, NJ), dtype=np.float32)
    xin_w[0:PW] = sw[:, :, :, ::-1].transpose(3, 1, 0, 2)
    xin_w = np.ascontiguousarray(xin_w).reshape(PIN, CW, XC).astype(NP_BF16)
    a_w = np.zeros((PIN, CW, PO), dtype=np.float32)
    a_w[0:PW] = a_mat[CD:].transpose(1, 0, 2)
    a_w = np.ascontiguousarray(a_w).astype(NP_BF16)

    return {"xin_d": xin_d, "a0_d": a0_d, "a1_d": a1_d,
            "xin_w": xin_w, "a_w": a_w}


def run(x, weight, bias, trace=False):
    nc = _get_nc()
    x = np.asarray(x, dtype=np.float32)
    weight = np.asarray(weight, dtype=np.float32)
    bias = np.asarray(bias, dtype=np.float32)
    in_maps = [_prep_core(x, weight, bias, core) for core in range(N_CORES)]
    res = run_bass_kernel_spmd(nc, in_maps, list(range(N_CORES)), trace=trace)

    y = np.empty((B, DIM, T), dtype=np.float32)
    for core in range(N_CORES):
        yp = res.results[core]["yout"].astype(np.float32)        # [PO, C, B*NJ]
        yc = yp.reshape(PO, C, B, NJ).transpose(2, 1, 3, 0)      # [B, C, j, m]
        y[:, core * C : (core + 1) * C, :TD] = yc.reshape(B, C, TD)

    wt = weight[:, 0, :]                                         # [DIM, K]
    # head outputs [0, 15) on host (dense-mode j=0 columns are garbage there)
    xh = np.concatenate(
        [np.zeros((B, DIM, K - 1), dtype=np.float32), x[:, :, : K - 1]], axis=2
    )
    xh = sliding_window_view(xh, K, axis=2)                      # [B,DIM,15,K]
    y[:, :, : K - 1] = np.einsum("bcmk,ck->bcm", xh, wt, optimize=True)
    # tail outputs [TD, T) on host: y[t] = sum_k w[k] x[t - 15 + k]
    xt = sliding_window_view(x[:, :, TD - K + 1 :], K, axis=2)   # [B,DIM,16,K]
    y[:, :, TD:] = np.einsum("bcmk,ck->bcm", xt, wt, optimize=True)

    y += bias[None, :, None]
    return y, res


def kernel(x, weight, bias):
    y, _ = run(
        np.asarray(x, dtype=np.float32),
        np.asarray(weight, dtype=np.float32),
        np.asarray(bias, dtype=np.float32),
    )
    return y


# revision 18
# speedup vs baseline: 1.1055x; 1.1055x over previous
"""Depthwise causal Conv1d (K=16) for x:(4, 2048, 8192) f32 on 8 TRN2 NeuronCores.

Strategy (tensor-parallel over channels, no cross-core communication):
  - Each core owns 256 channels (2048 / 8) for all 4 batches.
  - The time axis is cut into overlapping 127-sample windows with stride 112
    (15-sample causal halo), placed on SBUF partitions 0..126 and
    time-REVERSED within each window.  The depthwise conv of one channel is
    then a single banded-Toeplitz matmul on the TensorEngine:
        psum[m, (b,j)] = sum_p A[p, m] * X[p, (b,j)]
        A[p, m]   = w[126 - p - m]              for 111 <= p + m <= 126
        X[p, (b,j)] = x[b, c, 112*j + 111 - p]  (zero for t < 0)
        psum[m, (b,j)] = y[b, c, 112*j + m]
    Only NJ=73 full windows run on device (outputs [0, 8176)); the last 16
    samples per channel and the (identically zero) bias are done on the
    host, so no padded junk window is shipped.
  - All DRAM traffic is bf16 (inputs rounded on host, output upcast on
    host): the problem is HBM-bandwidth bound (16 DMA engines x 25.6 GB/s
    per core), and bf16 halves the bytes while staying ~0.4% rel err
    (budget is 2e-2).  Matmul runs in bf16, PSUM accumulates in f32.
  - DMA queues are dedicated: GpSimd = all loads (pure prefetch), Sync =
    all stores (compute-dependent), so prefetch never queues behind a
    store.  Loads/stores are split per half-chunk for finer pipelining,
    and the last chunks are tapered (16x15 + 8 + 8) to shorten the drain.
  - Epilogue: PSUM(f32) -> SBUF(bf16) converting copies, two channels per
    instruction, alternating Vector / Scalar engines.

The host does the sharding + window-layout transposes with numpy; the device
kernel sees only dense p-major arrays.
"""

import os
import sys

import numpy as np
from numpy.lib.stride_tricks import sliding_window_view

if "/opt/trn_rl_repo" not in sys.path:
    sys.path.insert(0, "/opt/trn_rl_repo")

import ml_dtypes

import concourse.bacc as bacc
import concourse.mybir as mybir
import concourse.tile as tile
from concourse.bass_utils import run_bass_kernel_spmd

F32 = mybir.dt.float32
BF16 = mybir.dt.bfloat16
NP_BF16 = np.dtype(ml_dtypes.bfloat16)
ACT_COPY = mybir.ActivationFunctionType.Copy

N_CORES = 8
B = 4             # batch
DIM = 2048        # channels
T = 8192          # time
K = 16            # conv taps
C = DIM // N_CORES    # channels per core = 256
PIN = 128         # matmul contraction rows (127-sample window + zero row)
PO = 112          # outputs per window (= 127 - 15), multiple of 16
NJ = 73           # full windows per (batch, channel); tail done on host
TD = NJ * PO          # device-computed samples per channel = 8176
XC = B * NJ           # x / out cols per channel = 292
PSB = 512             # psum bank stride (f32 elems); channel pair at 0 / PSB
CHUNKS = [16] * 15 + [8, 8]   # channels per device chunk (tapered tail)

_compiled_nc = None


def _build_kernel():
    nc = bacc.Bacc(None)

    xin = nc.declare_dram_parameter("xin", [PIN, C, XC], BF16, isOutput=False)
    a_in = nc.declare_dram_parameter("a_in", [PIN, C, PO], BF16, isOutput=False)
    yout = nc.declare_dram_parameter("yout", [PO, C, XC], BF16, isOutput=True)

    with tile.TileContext(nc) as tc:
        with (
            tc.tile_pool(name="xpool", bufs=7) as xpool,
            tc.tile_pool(name="apool", bufs=7) as apool,
            tc.tile_pool(name="opool", bufs=6) as opool,
            tc.tile_pool(name="psum", bufs=4, space="PSUM") as pspool,
        ):
            c0 = 0
            for ch in CHUNKS:
                x_t = xpool.tile([PIN, ch * XC], BF16)
                a_t = apool.tile([PIN, ch * PO], BF16)
                o_t = opool.tile([PO, ch * XC], BF16)
                n = ch * XC
                hf = ch // 2          # channels per half
                nh = hf * XC          # cols per half

                # loads (GpSimd queue)
                nc.gpsimd.dma_start(
                    out=x_t[:].rearrange("p (c j) -> p c j", c=ch),
                    in_=xin[:, c0 : c0 + ch, :],
                )
                nc.gpsimd.dma_start(
                    out=a_t[:].rearrange("p (c m) -> p c m", c=ch),
                    in_=a_in[:, c0 : c0 + ch, :],
                )

                for g in range(ch // 2):
                    ps = pspool.tile([PO, 2 * PSB], F32)
                    for h in range(2):
                        i = 2 * g + h
                        nc.tensor.matmul(
                            ps[:, h * PSB : h * PSB + XC],
                            a_t[:, i * PO : (i + 1) * PO],
                            x_t[:, i * XC : (i + 1) * XC],
                            start=True,
                            stop=True,
                        )
                    # converting psum(f32) -> sbuf(bf16) copy, 2 ch per inst
                    src = ps[:].rearrange("p (g q) -> p g q", g=2)[:, :, 0:XC]
                    dst = o_t[:, 2 * g * XC : (2 * g + 2) * XC].rearrange(
                        "p (g q) -> p g q", g=2
                    )
                    if g % 2 == 0:
                        nc.vector.tensor_copy(dst, src)
                    else:
                        nc.scalar.activation(dst, src, ACT_COPY)

                    # store (Sync queue) as soon as each half's copies land
                    if 2 * (g + 1) == hf:
                        nc.sync.dma_start(
                            out=yout[:, c0 : c0 + hf, :],
                            in_=o_t[:, 0:nh].rearrange("p (c j) -> p c j", c=hf),
                        )
                    elif 2 * (g + 1) == ch:
                        nc.sync.dma_start(
                            out=yout[:, c0 + hf : c0 + ch, :],
                            in_=o_t[:, nh:n].rearrange("p (c j) -> p c j", c=hf),
                        )
                c0 += ch

    nc.compile()
    return nc


def _get_nc():
    global _compiled_nc
    if _compiled_nc is None:
        _compiled_nc = _build_kernel()
    return _compiled_nc


def _prep_core(x, weight, bias, core):
    """Build the per-core input map (numpy only)."""
    cs = slice(core * C, (core + 1) * C)
    xs = x[:, cs, :]                       # [B, C, T]
    w = weight[cs, 0, :]                   # [C, K]

    # X[p, c, (b, j)] = x[b, c, 112*j + 111 - p] for p in [0, 127); row 127 = 0
    # xpad = [15 zeros] ++ x[:TD]; window j = xpad[112j : 112j + 127]
    PW = PIN - 1
    xpad = np.zeros((B, C, PO * (NJ - 1) + PW), dtype=np.float32)
    xpad[:, :, K - 1 :] = xs[:, :, :TD]
    sw = sliding_window_view(xpad, PW, axis=2)[:, :, ::PO, :]    # [B,C,NJ,127]
    xin = np.zeros((PIN, C, B, NJ), dtype=np.float32)
    xin[0:PW] = sw[:, :, :, ::-1].transpose(3, 1, 0, 2)
    xin = np.ascontiguousarray(xin).reshape(PIN, C, XC).astype(NP_BF16)

    # A[p, m] = w[126 - p - m] for 111 <= p + m <= 126, p in [0, 127); row 127 = 0
    idx = np.arange(PW)[:, None] + np.arange(PO)[None, :]    # p + m
    amask = (idx >= 111) & (idx <= 126)
    aidx = np.clip(126 - idx, 0, K - 1)
    a_mat = np.where(amask[None], w[:, aidx], 0.0)           # [C, 127, PO]
    a_in = np.zeros((PIN, C, PO), dtype=np.float32)
    a_in[0:PW] = a_mat.transpose(1, 0, 2)
    a_in = np.ascontiguousarray(a_in).astype(NP_BF16)

    return {"xin": xin, "a_in": a_in}


def run(x, weight, bias, trace=False):
    nc = _get_nc()
    x = np.asarray(x, dtype=np.float32)
    weight = np.asarray(weight, dtype=np.float32)
    bias = np.asarray(bias, dtype=np.float32)
    in_maps = [_prep_core(x, weight, bias, core) for core in range(N_CORES)]
    res = run_bass_kernel_spmd(nc, in_maps, list(range(N_CORES)), trace=trace)

    y = np.empty((B, DIM, T), dtype=np.float32)
    for core in range(N_CORES):
        yp = res.results[core]["yout"].astype(np.float32)        # [PO, C, B*NJ]
        yc = yp.reshape(PO, C, B, NJ).transpose(2, 1, 3, 0)      # [B, C, j, m]
        y[:, core * C : (core + 1) * C, :TD] = yc.reshape(B, C, TD)

    # tail outputs [TD, T) in f32 on host: y[t] = sum_k w[k] x[t - 15 + k]
    wt = weight[:, 0, :]                                         # [DIM, K]
    xt = sliding_window_view(x[:, :, TD - K + 1 :], K, axis=2)   # [B,DIM,16,K]
    y[:, :, TD:] = np.einsum("bcmk,ck->bcm", xt, wt, optimize=True)

    y += bias[None, :, None]
    return y, res


def kernel(x, weight, bias):
    y, _ = run(
        np.asarray(x, dtype=np.float32),
        np.asarray(weight, dtype=np.float32),
        np.asarray(bias, dtype=np.float32),
    )
    return y


# revision 19
# speedup vs baseline: 1.2190x; 1.1026x over previous
"""Depthwise causal Conv1d (K=16) for x:(4, 2048, 8192) f32 on 8 TRN2 NeuronCores.

Strategy (tensor-parallel over channels, no cross-core communication):
  - Each core owns 256 channels (2048 / 8) for all 4 batches.
  - The time axis is cut into overlapping 127-sample windows with stride 112
    (15-sample causal halo), placed on SBUF partitions 0..126 and
    time-REVERSED within each window.  The depthwise conv of one channel is
    then a single banded-Toeplitz matmul on the TensorEngine:
        psum[m, (b,j)] = sum_p A[p, m] * X[p, (b,j)]
        A[p, m]   = w[126 - p - m]              for 111 <= p + m <= 126
        X[p, (b,j)] = x[b, c, 112*j + 111 - p]  (zero for t < 0)
        psum[m, (b,j)] = y[b, c, 112*j + m]
    Only NJ=73 full windows run on device (outputs [0, 8176)); the last 16
    samples per channel and the (identically zero) bias are done on the
    host, so no padded junk window is shipped.
  - All DRAM traffic is bf16 (inputs rounded on host, output upcast on
    host): the problem is HBM-bandwidth bound (16 DMA engines x 25.6 GB/s
    per core), and bf16 halves the bytes while staying ~0.4% rel err
    (budget is 2e-2).  Matmul runs in bf16, PSUM accumulates in f32.
  - DMA queues are dedicated: GpSimd = all loads (pure prefetch), Sync =
    all stores (compute-dependent), so prefetch never queues behind a
    store.  Loads/stores are split per half-chunk for finer pipelining,
    and the last chunks are tapered (16x15 + 8 + 8) to shorten the drain.
  - Epilogue: PSUM(f32) -> SBUF(bf16) converting copies, two channels per
    instruction, alternating Vector / Scalar engines.

The host does the sharding + window-layout transposes with numpy; the device
kernel sees only dense p-major arrays.
"""

import os
import sys

import numpy as np
from numpy.lib.stride_tricks import sliding_window_view

if "/opt/trn_rl_repo" not in sys.path:
    sys.path.insert(0, "/opt/trn_rl_repo")

import ml_dtypes

import concourse.bacc as bacc
import concourse.mybir as mybir
import concourse.tile as tile
from concourse.bass_utils import run_bass_kernel_spmd

F32 = mybir.dt.float32
BF16 = mybir.dt.bfloat16
NP_BF16 = np.dtype(ml_dtypes.bfloat16)
ACT_COPY = mybir.ActivationFunctionType.Copy

N_CORES = 8
B = 4             # batch
DIM = 2048        # channels
T = 8192          # time
K = 16            # conv taps
C = DIM // N_CORES    # channels per core = 256
PIN = 128         # matmul contraction rows (127-sample window + zero row)
PO = 112          # outputs per window (= 127 - 15), multiple of 16
NJ = 73           # full windows per (batch, channel); tail done on host
TD = NJ * PO          # device-computed samples per channel = 8176
XC = B * NJ           # x / out cols per channel = 292
PSB = 512             # psum bank stride (f32 elems); channel pair at 0 / PSB
CHUNKS = [16] * 15 + [8, 8]   # channels per device chunk (tapered tail)

_compiled_nc = None


def _build_kernel():
    nc = bacc.Bacc(None)

    xin = nc.declare_dram_parameter("xin", [PIN, C, XC], BF16, isOutput=False)
    a_in = nc.declare_dram_parameter("a_in", [PIN, C, PO], BF16, isOutput=False)
    yout = nc.declare_dram_parameter("yout", [PO, C, XC], BF16, isOutput=True)

    with tile.TileContext(nc) as tc:
        with (
            tc.tile_pool(name="xpool", bufs=7) as xpool,
            tc.tile_pool(name="apool", bufs=7) as apool,
            tc.tile_pool(name="opool", bufs=6) as opool,
            tc.tile_pool(name="psum", bufs=4, space="PSUM") as pspool,
        ):
            c0 = 0
            for ch in CHUNKS:
                x_t = xpool.tile([PIN, ch * XC], BF16)
                a_t = apool.tile([PIN, ch * PO], BF16)
                o_t = opool.tile([PO, ch * XC], BF16)
                n = ch * XC
                hf = ch // 2          # channels per half
                nh = hf * XC          # cols per half

                # loads (GpSimd queue): x half 0, A, x half 1 — the first
                # matmul pair needs only x half 0 + A, so it starts earlier
                nc.gpsimd.dma_start(
                    out=x_t[:, 0:nh].rearrange("p (c j) -> p c j", c=hf),
                    in_=xin[:, c0 : c0 + hf, :],
                )
                nc.gpsimd.dma_start(
                    out=a_t[:].rearrange("p (c m) -> p c m", c=ch),
                    in_=a_in[:, c0 : c0 + ch, :],
                )
                nc.gpsimd.dma_start(
                    out=x_t[:, nh:n].rearrange("p (c j) -> p c j", c=hf),
                    in_=xin[:, c0 + hf : c0 + ch, :],
                )

                for g in range(ch // 2):
                    ps = pspool.tile([PO, 2 * PSB], F32)
                    for h in range(2):
                        i = 2 * g + h
                        nc.tensor.matmul(
                            ps[:, h * PSB : h * PSB + XC],
                            a_t[:, i * PO : (i + 1) * PO],
                            x_t[:, i * XC : (i + 1) * XC],
                            start=True,
                            stop=True,
                        )
                    # converting psum(f32) -> sbuf(bf16) copy, 2 ch per inst
                    src = ps[:].rearrange("p (g q) -> p g q", g=2)[:, :, 0:XC]
                    dst = o_t[:, 2 * g * XC : (2 * g + 2) * XC].rearrange(
                        "p (g q) -> p g q", g=2
                    )
                    if g % 2 == 0:
                        nc.vector.tensor_copy(dst, src)
                    else:
                        nc.scalar.activation(dst, src, ACT_COPY)

                    # store (Sync queue) as soon as each half's copies land
                    if 2 * (g + 1) == hf:
                        nc.sync.dma_start(
                            out=yout[:, c0 : c0 + hf, :],
                            in_=o_t[:, 0:nh].rearrange("p (c j) -> p c j", c=hf),
                        )
                    elif 2 * (g + 1) == ch:
                        nc.sync.dma_start(
                            out=yout[:, c0 + hf : c0 + ch, :],
                            in_=o_t[:, nh:n].rearrange("p (c j) -> p c j", c=hf),
                        )
                c0 += ch

    nc.compile()
    return nc


def _get_nc():
    global _compiled_nc
    if _compiled_nc is None:
        _compiled_nc = _build_kernel()
    return _compiled_nc


def _prep_core(x, weight, bias, core):
    """Build the per-core input map (numpy only)."""
    cs = slice(core * C, (core + 1) * C)
    xs = x[:, cs, :]                       # [B, C, T]
    w = weight[cs, 0, :]                   # [C, K]

    # X[p, c, (b, j)] = x[b, c, 112*j + 111 - p] for p in [0, 127); row 127 = 0
    # xpad = [15 zeros] ++ x[:TD]; window j = xpad[112j : 112j + 127]
    PW = PIN - 1
    xpad = np.zeros((B, C, PO * (NJ - 1) + PW), dtype=np.float32)
    xpad[:, :, K - 1 :] = xs[:, :, :TD]
    sw = sliding_window_view(xpad, PW, axis=2)[:, :, ::PO, :]    # [B,C,NJ,127]
    xin = np.zeros((PIN, C, B, NJ), dtype=np.float32)
    xin[0:PW] = sw[:, :, :, ::-1].transpose(3, 1, 0, 2)
    xin = np.ascontiguousarray(xin).reshape(PIN, C, XC).astype(NP_BF16)

    # A[p, m] = w[126 - p - m] for 111 <= p + m <= 126, p in [0, 127); row 127 = 0
    idx = np.arange(PW)[:, None] + np.arange(PO)[None, :]    # p + m
    amask = (idx >= 111) & (idx <= 126)
    aidx = np.clip(126 - idx, 0, K - 1)
    a_mat = np.where(amask[None], w[:, aidx], 0.0)           # [C, 127, PO]
    a_in = np.zeros((PIN, C, PO), dtype=np.float32)
    a_in[0:PW] = a_mat.transpose(1, 0, 2)
    a_in = np.ascontiguousarray(a_in).astype(NP_BF16)

    return {"xin": xin, "a_in": a_in}


def run(x, weight, bias, trace=False):
    nc = _get_nc()
    x = np.asarray(x, dtype=np.float32)
    weight = np.asarray(weight, dtype=np.float32)
    bias = np.asarray(bias, dtype=np.float32)
    in_maps = [_prep_core(x, weight, bias, core) for core in range(N_CORES)]
    res = run_bass_kernel_spmd(nc, in_maps, list(range(N_CORES)), trace=trace)

    y = np.empty((B, DIM, T), dtype=np.float32)
    for core in range(N_CORES):
        yp = res.results[core]["yout"].astype(np.float32)        # [PO, C, B*NJ]
        yc = yp.reshape(PO, C, B, NJ).transpose(2, 1, 3, 0)      # [B, C, j, m]
        y[:, core * C : (core + 1) * C, :TD] = yc.reshape(B, C, TD)

    # tail outputs [TD, T) in f32 on host: y[t] = sum_k w[k] x[t - 15 + k]
    wt = weight[:, 0, :]                                         # [DIM, K]
    xt = sliding_window_view(x[:, :, TD - K + 1 :], K, axis=2)   # [B,DIM,16,K]
    y[:, :, TD:] = np.einsum("bcmk,ck->bcm", xt, wt, optimize=True)

    y += bias[None, :, None]
    return y, res


def kernel(x, weight, bias):
    y, _ = run(
        np.asarray(x, dtype=np.float32),
        np.asarray(weight, dtype=np.float32),
        np.asarray(bias, dtype=np.float32),
    )
    return y
